# revision 40
# baseline (speedup 1.0000x reference)
"""Trainium2 Bass kernel for nn_PredictAverageReward.

Per core (fruits sharded 8 ways, 512 fruits each):
  1. fp32 GEMM chain on TensorE produces Rd [512 fruits, 256 tools] laid out
     as one SBUF tile rd_all [128, 4*256] (4 fruit blocks along free axis).
     min_r is dropped: it cancels in all comparisons and the output depends
     only on comparisons.
  2. Pairwise-win counting. For each column j we need
     ge[f, i] = (Rd[f, i] >= Rd[f, j]) for i < j, then counts = sum_f ge.
     Compares are split across three engines:
       - VectorE merged tensor_tensor with a stride-0 broadcast comparand
         (one instruction covers all 4 fruit blocks) for small/mid j,
       - VectorE tensor_scalar         (4 instrs/j) for mid j,
       - GpSimd  tensor_scalar         (4 instrs/j) for a mid slice,
       - ScalarE Sign activation       (4 instrs/j, values in {-1,0,1}) for
         large j.
     ge is written as bf16 (exact for 0/±1) so the reduction matmuls skip the
     fp32 LOW/HIGH split. Reduction: one bf16 matmul per (j, block) whose
     stationary operand is a [128, 32] one-hot column routing the sums into
     PSUM row j%128 of col-group (j%128)//32; all rows accumulate into two
     pre-zeroed PSUM banks (all start=False), drained with two copies.
  3. Host: sum per-core counts, threshold at 2048, build the proposal.
"""

import sys

for _p in ("/opt/trn_rl_repo",):
    if _p not in sys.path:
        sys.path.insert(0, _p)

import numpy as np

import concourse.bass as bass
import concourse.bacc as bacc
import concourse.mybir as mybir
import concourse.tile as tile
from concourse.bass_utils import run_bass_kernel_spmd

F32 = mybir.dt.float32
BF16 = mybir.dt.bfloat16

N_CORES = 8
N_FRUITS, N_TOOLS, P_F, P_T, D = 8192, 1024, 128, 128, 512
K_DOMAIN, BATCH = 256, 4096
F_PER_CORE = BATCH // N_CORES          # 512 fruits per core
N_FT = F_PER_CORE // 128               # 4 fruit blocks

# engine split over j (tunable):
T_GPS_LO, T_GPS_HI = 0, 0       # [lo, hi): GpSimd tensor_scalar (off: too slow)
T_ACT = 198                     # j >= T_ACT: ScalarE Sign  (sign-sum rows)
T_TT = 198                      # j < T_TT: DVE merged tensor_tensor

TRACE = False
LAST_RESULTS = None


def _j_engine(j):
    """-> 'tt' (DVE merged), 'ts' (DVE tensor_scalar), 'gps', 'act'"""
    if T_GPS_LO <= j < T_GPS_HI:
        return "gps"
    if j >= T_ACT:
        return "act"
    if j < T_TT:
        return "tt"
    return "ts"


def _sign_rows():
    return set(j for j in range(1, 256) if _j_engine(j) == "act")


def _build_nc():
    nc = bacc.Bacc()
    K = K_DOMAIN

    g_t = nc.dram_tensor("g_t", [128, F_PER_CORE], F32, kind="ExternalInput")
    mf = nc.dram_tensor("mf", [P_F, D], F32, kind="ExternalInput")
    wt_in = nc.dram_tensor("wt_in", [D, K], F32, kind="ExternalInput")
    counts = nc.dram_tensor("counts", [128, 2 * K], F32, kind="ExternalOutput")

    with tile.TileContext(nc) as tc:
        with (
            tc.tile_pool(name="persist", bufs=1) as pp,
            tc.tile_pool(name="ge", bufs=20) as gep,
            tc.tile_pool(name="ge1", bufs=48) as gact_pool,
            tc.tile_pool(name="gemm_ps", bufs=3, space=bass.MemorySpace.PSUM) as gps_pool,
            tc.tile_pool(name="cnt_ps", bufs=2, space=bass.MemorySpace.PSUM) as cps,
        ):
            # ---- constants ----
            # strip[:, 31] = 1 else 0; strip[:, 31-m:63-m] is a [128, 32]
            # one-hot-column matrix selecting row m of a 32-wide col-group.
            strip = pp.tile([128, 63], BF16, tag="strip")
            nc.gpsimd.memset(strip[:], 0.0)
            nc.gpsimd.memset(strip[:, 31:32], 1.0)
            zw = pp.tile([128, 128], BF16, tag="zw")
            nc.gpsimd.memset(zw[:], 0.0)
            zsb = pp.tile([128, K], BF16, tag="zsb")
            nc.gpsimd.memset(zsb[:], 0.0)

            # ---- load inputs ----
            gt_sb = pp.tile([128, F_PER_CORE], F32, tag="gt")
            nc.sync.dma_start(gt_sb[:], g_t[:])
            mf_sb = pp.tile([128, D], F32, tag="mf")
            nc.sync.dma_start(mf_sb[:], mf[:])
            # WT = ((tools_prop[domain_t] @ M_tool) @ M).T precomputed on host
            wt_sb = []
            for dt in range(4):
                t = pp.tile([128, K], F32, tag=f"wt{dt}")
                nc.sync.dma_start(t[:], wt_in[dt * 128:(dt + 1) * 128, :])
                wt_sb.append(t)

            # ---- fp32 GEMM chain ----
            pmfT_sb = []
            for dt in range(4):
                ps = gps_pool.tile([128, F_PER_CORE], F32)
                nc.tensor.matmul(ps[:], mf_sb[:, dt * 128:(dt + 1) * 128],
                                 gt_sb[:], start=True, stop=True)
                t = pp.tile([128, F_PER_CORE], F32, tag=f"pmfT{dt}")
                if dt % 2 == 0:
                    nc.scalar.copy(t[:], ps[:])
                else:
                    nc.vector.tensor_copy(t[:], ps[:])
                pmfT_sb.append(t)

            # Rd blocks stay resident in PSUM: ScalarE compares read PSUM
            # (cheaper init than SBUF); DVE compares read the SBUF copy.
            rd_all = pp.tile([128, N_FT * K], F32, tag="rd")
            neg_all = pp.tile([128, N_FT * K], F32, tag="neg")
            for ft in range(N_FT):
                ps = gps_pool.tile([128, K], F32)
                for dt in range(4):
                    nc.tensor.matmul(ps[:], pmfT_sb[dt][:, ft * 128:(ft + 1) * 128],
                                     wt_sb[dt][:], start=(dt == 0), stop=(dt == 3))
                nc.vector.tensor_copy(rd_all[:, ft * K:(ft + 1) * K], ps[:])
                if ft % 2 == 0:
                    nc.scalar.mul(neg_all[:, ft * K:(ft + 1) * K], ps[:], -1.0)
                else:
                    nc.vector.tensor_scalar_mul(neg_all[:, ft * K:(ft + 1) * K],
                                                ps[:], -1.0)
            rd3 = rd_all[:].rearrange("p (b i) -> p b i", b=N_FT)

            # ---- pairwise-win counting ----
            cntA = cps.tile([128, K], F32)   # j in [0, 128)   -> row j
            cntB = cps.tile([128, K], F32)   # j in [128, 256) -> row j-128
            nc.tensor.matmul(cntA[:], zw[:], zsb[:], start=True, stop=False)
            nc.tensor.matmul(cntB[:], zw[:], zsb[:], start=True, stop=False)

            def cnt_mm(j, rhs):
                jj = j % 128
                c, m = jj // 32, jj % 32
                dst = cntA if j < 128 else cntB
                nc.tensor.matmul(dst[c * 32:(c + 1) * 32, 0:rhs.shape[-1]],
                                 strip[:, 31 - m:63 - m], rhs,
                                 start=False, stop=False, tile_position=(0, c * 32))

            def act_cmp(j, ft):
                L = j + (j & 1)
                ge = gact_pool.tile([128, K], BF16, tag="ge1")
                nc.scalar.activation(
                    ge[:, 0:L], rd_all[:, ft * K:ft * K + L],
                    mybir.ActivationFunctionType.Sign,
                    bias=neg_all[:, ft * K + j:ft * K + j + 1], scale=1.0)
                return ge

            # Global interleave over all j: col-groups rotate so consecutive
            # reduction matmuls hit distinct PE col-groups and overlap.  ACT
            # j's are woven into the early/middle slots only (1 per 3 DVE js)
            # so the just-in-time ACT compares never pace the kernel tail.
            base = sorted(range(1, 256), key=lambda j: (j % 32, j // 32))
            dve_order = [j for j in base if _j_engine(j) != "act"]
            act_order = [j for j in base if _j_engine(j) == "act"]
            order = []
            di = ai = 0
            while di < len(dve_order) or ai < len(act_order):
                # weave ACT js densely enough that they exhaust ~30 slots
                # before the end: ratio 3 early, 2 for the later ACT js
                step = 3 if ai < 40 else 2
                for _ in range(step):
                    if di < len(dve_order):
                        order.append(dve_order[di]); di += 1
                if ai < len(act_order):
                    order.append(act_order[ai]); ai += 1

            # Pre-buffer ScalarE compares (blocks 0/1 of the first ACT j's):
            # a (j, ft) compare only needs Rd block ft, so ACT starts while
            # the GEMM is still producing later blocks and stays a few
            # instructions ahead of PE's consumption afterwards.  Reduction
            # matmuls are never hoisted.
            hoist = set(act_order[:16])
            act_ge = {}
            for ft in range(2):
                for j in act_order:
                    if j in hoist:
                        act_ge[(j, ft)] = act_cmp(j, ft)

            # Fill DVE's idle window (while the GEMM finishes blocks 1-3)
            # with block-0 tensor_scalar compares for the last DVE js; their
            # slot then only runs a 3-block tensor_tensor.
            dve_hoist = set(dve_order[-24:])
            dve_ge0 = {}
            for j in sorted(dve_hoist):
                L = j + (j & 1)
                ge = gact_pool.tile([128, K], BF16, tag="ge0")
                nc.vector.tensor_scalar(ge[:, 0:L], rd_all[:, 0:L],
                                        rd_all[:, j:j + 1], None,
                                        mybir.AluOpType.is_ge)
                dve_ge0[j] = ge

            for j in order:
                L = j + (j & 1)          # even free-dim keeps DVE 2x mode
                if _j_engine(j) == "act":
                    for ft in range(N_FT):
                        if (j, ft) in act_ge:
                            ge = act_ge.pop((j, ft))
                        else:
                            ge = act_cmp(j, ft)
                        cnt_mm(j, ge[:, 0:L])
                    continue
                if j in dve_hoist:
                    ge = gep.tile([128, N_FT * K], BF16, tag="ge")
                    nb = N_FT - 1
                    in0 = rd3[:, 1:, 0:L]
                    in1 = rd3[:, 1:, j:j + 1].broadcast_to((128, nb, L))
                    out = ge[:, 0:nb * L].rearrange("p (b i) -> p b i", b=nb)
                    nc.vector.tensor_tensor(out, in0, in1, mybir.AluOpType.is_ge)
                    cnt_mm(j, dve_ge0.pop(j)[:, 0:L])
                    for bi in range(nb):
                        cnt_mm(j, ge[:, bi * L:(bi + 1) * L])
                    continue
                ge = gep.tile([128, N_FT * K], BF16, tag="ge")
                in0 = rd3[:, :, 0:L]
                in1 = rd3[:, :, j:j + 1].broadcast_to((128, N_FT, L))
                out = ge[:, 0:N_FT * L].rearrange("p (b i) -> p b i", b=N_FT)
                nc.vector.tensor_tensor(out, in0, in1, mybir.AluOpType.is_ge)
                for ft in range(N_FT):
                    cnt_mm(j, ge[:, ft * L:(ft + 1) * L])

            # close both accumulation groups across all 128 partitions
            nc.tensor.matmul(cntA[:], zw[:], zsb[:], start=False, stop=True)
            nc.tensor.matmul(cntB[:], zw[:], zsb[:], start=False, stop=True)

            out_sb = pp.tile([128, 2 * K], F32, tag="out")
            nc.scalar.copy(out_sb[:, 0:K], cntA[:])
            nc.scalar.copy(out_sb[:, K:2 * K], cntB[:])
            nc.sync.dma_start(counts[:], out_sb[:])

    nc.compile()
    return nc


def _host_inputs(fruits_prop, tools_prop, M_fruit, M_tool, M, domain_f, domain_t):
    G = np.ascontiguousarray(np.asarray(fruits_prop, np.float32)[np.asarray(domain_f, np.int64)])
    toolsD = np.asarray(tools_prop, np.float32)[np.asarray(domain_t, np.int64)]
    mf = np.ascontiguousarray(np.asarray(M_fruit, np.float32))
    mt = np.asarray(M_tool, np.float32)
    m = np.asarray(M, np.float32)
    WT = np.ascontiguousarray(((toolsD @ mt) @ m).T)   # [512, 256]
    in_maps = []
    for c in range(N_CORES):
        Gc = G[c * F_PER_CORE:(c + 1) * F_PER_CORE]
        in_maps.append({
            "g_t": np.ascontiguousarray(Gc.T),
            "mf": mf,
            "wt_in": WT,
        })
    return in_maps


def decode_counts(counts_sum):
    """counts_sum: [128, 512] summed over cores -> C[i, j] win counts (i < j)."""
    sign_rows = _sign_rows()
    C = np.zeros((K_DOMAIN, K_DOMAIN), np.float64)
    for j in range(1, K_DOMAIN):
        half = j // 128
        row = counts_sum[j % 128, half * K_DOMAIN: half * K_DOMAIN + j]
        if j in sign_rows:
            C[:j, j] = np.rint((row + BATCH) / 2.0)
        else:
            C[:j, j] = np.rint(row)
    return C


def _predict(C, domain_t, tools_labels):
    pos = np.full(N_TOOLS, -1, np.int64)
    pos[np.asarray(domain_t, np.int64)] = np.arange(K_DOMAIN)
    l1 = pos[np.asarray(tools_labels[0], np.int64)]
    l2 = pos[np.asarray(tools_labels[1], np.int64)]
    B = l1.shape[0]
    half = BATCH // 2
    choice = np.ones(B, np.int64)
    lt = (l1 >= 0) & (l2 >= 0) & (l1 < l2)
    gt = (l1 >= 0) & (l2 >= 0) & (l1 > l2)
    choice[lt] = np.where(C[l1[lt], l2[lt]] >= half, 0, 1)
    choice[gt] = np.where(C[l2[gt], l1[gt]] <= half, 0, 1)
    out = np.zeros((B, 4), np.float32)
    out[:, 0] = 1.0
    out[np.arange(B), 1 + choice] = 1.0
    return out


def kernel(fruits_prop, tools_prop, M_fruit, M_tool, M, min_r, domain_f,
           domain_t, tools_labels):
    global LAST_RESULTS
    in_maps = _host_inputs(fruits_prop, tools_prop, M_fruit, M_tool, M,
                           domain_f, domain_t)
    nc = _build_nc()
    res = run_bass_kernel_spmd(nc, in_maps, list(range(N_CORES)), trace=TRACE)
    LAST_RESULTS = res
    counts_sum = np.zeros((128, 2 * K_DOMAIN), np.float64)
    for c in range(N_CORES):
        counts_sum += res.results[c]["counts"].astype(np.float64)
    C = decode_counts(counts_sum)
    return _predict(C, domain_t, tools_labels)


# revision 41
# speedup vs baseline: 1.0145x; 1.0145x over previous
"""Trainium2 Bass kernel for nn_PredictAverageReward.

Per core (fruits sharded 8 ways, 512 fruits each):
  1. fp32 GEMM chain on TensorE produces Rd [512 fruits, 256 tools] laid out
     as one SBUF tile rd_all [128, 4*256] (4 fruit blocks along free axis).
     min_r is dropped: it cancels in all comparisons and the output depends
     only on comparisons.
  2. Pairwise-win counting. For each column j we need
     ge[f, i] = (Rd[f, i] >= Rd[f, j]) for i < j, then counts = sum_f ge.
     Compares are split across three engines:
       - VectorE merged tensor_tensor with a stride-0 broadcast comparand
         (one instruction covers all 4 fruit blocks) for small/mid j,
       - VectorE tensor_scalar         (4 instrs/j) for mid j,
       - GpSimd  tensor_scalar         (4 instrs/j) for a mid slice,
       - ScalarE Sign activation       (4 instrs/j, values in {-1,0,1}) for
         large j.
     ge is written as bf16 (exact for 0/±1) so the reduction matmuls skip the
     fp32 LOW/HIGH split. Reduction: one bf16 matmul per (j, block) whose
     stationary operand is a [128, 32] one-hot column routing the sums into
     PSUM row j%128 of col-group (j%128)//32; all rows accumulate into two
     pre-zeroed PSUM banks (all start=False), drained with two copies.
  3. Host: sum per-core counts, threshold at 2048, build the proposal.
"""

import sys

for _p in ("/opt/trn_rl_repo",):
    if _p not in sys.path:
        sys.path.insert(0, _p)

import numpy as np

import concourse.bass as bass
import concourse.bacc as bacc
import concourse.mybir as mybir
import concourse.tile as tile
from concourse.bass_utils import run_bass_kernel_spmd

F32 = mybir.dt.float32
BF16 = mybir.dt.bfloat16

N_CORES = 8
N_FRUITS, N_TOOLS, P_F, P_T, D = 8192, 1024, 128, 128, 512
K_DOMAIN, BATCH = 256, 4096
F_PER_CORE = BATCH // N_CORES          # 512 fruits per core
N_FT = F_PER_CORE // 128               # 4 fruit blocks

# engine split over j (tunable):
T_GPS_LO, T_GPS_HI = 0, 0       # [lo, hi): GpSimd tensor_scalar (off: too slow)
T_ACT = 196                     # j >= T_ACT: ScalarE Sign  (sign-sum rows)
T_TT = 196                      # j < T_TT: DVE merged tensor_tensor

TRACE = False
LAST_RESULTS = None


def _j_engine(j):
    """-> 'tt' (DVE merged), 'ts' (DVE tensor_scalar), 'gps', 'act'"""
    if T_GPS_LO <= j < T_GPS_HI:
        return "gps"
    if j >= T_ACT:
        return "act"
    if j < T_TT:
        return "tt"
    return "ts"


def _sign_rows():
    return set(j for j in range(1, 256) if _j_engine(j) == "act")


def _build_nc():
    nc = bacc.Bacc()
    K = K_DOMAIN

    g_t = nc.dram_tensor("g_t", [128, F_PER_CORE], F32, kind="ExternalInput")
    mf = nc.dram_tensor("mf", [P_F, D], F32, kind="ExternalInput")
    wt_in = nc.dram_tensor("wt_in", [D, K], F32, kind="ExternalInput")
    counts = nc.dram_tensor("counts", [128, 2 * K], F32, kind="ExternalOutput")

    with tile.TileContext(nc) as tc:
        with (
            tc.tile_pool(name="persist", bufs=1) as pp,
            tc.tile_pool(name="ge", bufs=16) as gep,
            tc.tile_pool(name="ge1", bufs=48) as gact_pool,
            tc.tile_pool(name="gemm_ps", bufs=3, space=bass.MemorySpace.PSUM) as gps_pool,
            tc.tile_pool(name="cnt_ps", bufs=2, space=bass.MemorySpace.PSUM) as cps,
        ):
            # ---- constants ----
            # strip[:, 31] = 1 else 0; strip[:, 31-m:63-m] is a [128, 32]
            # one-hot-column matrix selecting row m of a 32-wide col-group.
            strip = pp.tile([128, 63], BF16, tag="strip")
            nc.gpsimd.memset(strip[:], 0.0)
            nc.gpsimd.memset(strip[:, 31:32], 1.0)
            zw = pp.tile([128, 128], BF16, tag="zw")
            nc.gpsimd.memset(zw[:], 0.0)
            zsb = pp.tile([128, K], BF16, tag="zsb")
            nc.gpsimd.memset(zsb[:], 0.0)

            # ---- load inputs ----
            gt_sb = pp.tile([128, F_PER_CORE], F32, tag="gt")
            nc.sync.dma_start(gt_sb[:], g_t[:])
            mf_sb = pp.tile([128, D], F32, tag="mf")
            nc.sync.dma_start(mf_sb[:], mf[:])
            # WT = ((tools_prop[domain_t] @ M_tool) @ M).T precomputed on host
            wt_sb = []
            for dt in range(4):
                t = pp.tile([128, K], F32, tag=f"wt{dt}")
                nc.sync.dma_start(t[:], wt_in[dt * 128:(dt + 1) * 128, :])
                wt_sb.append(t)

            # ---- fp32 GEMM chain ----
            pmfT_sb = []
            for dt in range(4):
                ps = gps_pool.tile([128, F_PER_CORE], F32)
                nc.tensor.matmul(ps[:], mf_sb[:, dt * 128:(dt + 1) * 128],
                                 gt_sb[:], start=True, stop=True)
                t = pp.tile([128, F_PER_CORE], F32, tag=f"pmfT{dt}")
                if dt % 2 == 0:
                    nc.scalar.copy(t[:], ps[:])
                else:
                    nc.vector.tensor_copy(t[:], ps[:])
                pmfT_sb.append(t)

            # Rd blocks stay resident in PSUM: ScalarE compares read PSUM
            # (cheaper init than SBUF); DVE compares read the SBUF copy.
            rd_all = pp.tile([128, N_FT * K], F32, tag="rd")
            neg_all = pp.tile([128, N_FT * K], F32, tag="neg")
            for ft in range(N_FT):
                ps = gps_pool.tile([128, K], F32)
                for dt in range(4):
                    nc.tensor.matmul(ps[:], pmfT_sb[dt][:, ft * 128:(ft + 1) * 128],
                                     wt_sb[dt][:], start=(dt == 0), stop=(dt == 3))
                nc.vector.tensor_copy(rd_all[:, ft * K:(ft + 1) * K], ps[:])
                if ft % 2 == 0:
                    nc.scalar.mul(neg_all[:, ft * K:(ft + 1) * K], ps[:], -1.0)
                else:
                    nc.vector.tensor_scalar_mul(neg_all[:, ft * K:(ft + 1) * K],
                                                ps[:], -1.0)
            rd3 = rd_all[:].rearrange("p (b i) -> p b i", b=N_FT)

            # ---- pairwise-win counting ----
            cntA = cps.tile([128, K], F32)   # j in [0, 128)   -> row j
            cntB = cps.tile([128, K], F32)   # j in [128, 256) -> row j-128
            nc.tensor.matmul(cntA[:], zw[:], zsb[:], start=True, stop=False)
            nc.tensor.matmul(cntB[:], zw[:], zsb[:], start=True, stop=False)

            def cnt_mm(j, rhs):
                jj = j % 128
                c, m = jj // 32, jj % 32
                dst = cntA if j < 128 else cntB
                nc.tensor.matmul(dst[c * 32:(c + 1) * 32, 0:rhs.shape[-1]],
                                 strip[:, 31 - m:63 - m], rhs,
                                 start=False, stop=False, tile_position=(0, c * 32))

            def act_cmp(j, ft):
                L = j + (j & 1)
                ge = gact_pool.tile([128, K], BF16, tag="ge1")
                nc.scalar.activation(
                    ge[:, 0:L], rd_all[:, ft * K:ft * K + L],
                    mybir.ActivationFunctionType.Sign,
                    bias=neg_all[:, ft * K + j:ft * K + j + 1], scale=1.0)
                return ge

            # Global interleave over all j: col-groups rotate so consecutive
            # reduction matmuls hit distinct PE col-groups and overlap.  ACT
            # j's are woven into the early/middle slots only (1 per 3 DVE js)
            # so the just-in-time ACT compares never pace the kernel tail.
            base = sorted(range(1, 256), key=lambda j: (j % 32, j // 32))
            dve_order = [j for j in base if _j_engine(j) != "act"]
            act_order = [j for j in base if _j_engine(j) == "act"]
            order = []
            di = ai = 0
            while di < len(dve_order) or ai < len(act_order):
                # weave ACT js densely enough that they exhaust ~30 slots
                # before the end: ratio 3 early, 2 for the later ACT js
                step = 3 if ai < 40 else 2
                for _ in range(step):
                    if di < len(dve_order):
                        order.append(dve_order[di]); di += 1
                if ai < len(act_order):
                    order.append(act_order[ai]); ai += 1

            # Pre-buffer ScalarE compares (blocks 0/1 of the first ACT j's):
            # a (j, ft) compare only needs Rd block ft, so ACT starts while
            # the GEMM is still producing later blocks and stays a few
            # instructions ahead of PE's consumption afterwards.  Reduction
            # matmuls are never hoisted.
            hoist = set(act_order[:16])
            act_ge = {}
            for ft in range(2):
                for j in act_order:
                    if j in hoist:
                        act_ge[(j, ft)] = act_cmp(j, ft)

            # Fill DVE's idle window (while the GEMM finishes blocks 1-3)
            # with block-0 tensor_scalar compares for the last DVE js; their
            # slot then only runs a 3-block tensor_tensor.
            dve_hoist = set(dve_order[-16:])
            dve_ge0 = {}
            for j in sorted(dve_hoist):
                L = j + (j & 1)
                ge = gact_pool.tile([128, K], BF16, tag="ge0")
                nc.vector.tensor_scalar(ge[:, 0:L], rd_all[:, 0:L],
                                        rd_all[:, j:j + 1], None,
                                        mybir.AluOpType.is_ge)
                dve_ge0[j] = ge

            for j in order:
                L = j + (j & 1)          # even free-dim keeps DVE 2x mode
                if _j_engine(j) == "act":
                    for ft in range(N_FT):
                        if (j, ft) in act_ge:
                            ge = act_ge.pop((j, ft))
                        else:
                            ge = act_cmp(j, ft)
                        cnt_mm(j, ge[:, 0:L])
                    continue
                if j in dve_hoist:
                    ge = gep.tile([128, N_FT * K], BF16, tag="ge")
                    nb = N_FT - 1
                    in0 = rd3[:, 1:, 0:L]
                    in1 = rd3[:, 1:, j:j + 1].broadcast_to((128, nb, L))
                    out = ge[:, 0:nb * L].rearrange("p (b i) -> p b i", b=nb)
                    nc.vector.tensor_tensor(out, in0, in1, mybir.AluOpType.is_ge)
                    cnt_mm(j, dve_ge0.pop(j)[:, 0:L])
                    for bi in range(nb):
                        cnt_mm(j, ge[:, bi * L:(bi + 1) * L])
                    continue
                ge = gep.tile([128, N_FT * K], BF16, tag="ge")
                in0 = rd3[:, :, 0:L]
                in1 = rd3[:, :, j:j + 1].broadcast_to((128, N_FT, L))
                out = ge[:, 0:N_FT * L].rearrange("p (b i) -> p b i", b=N_FT)
                nc.vector.tensor_tensor(out, in0, in1, mybir.AluOpType.is_ge)
                for ft in range(N_FT):
                    cnt_mm(j, ge[:, ft * L:(ft + 1) * L])

            # close both accumulation groups across all 128 partitions
            nc.tensor.matmul(cntA[:], zw[:], zsb[:], start=False, stop=True)
            nc.tensor.matmul(cntB[:], zw[:], zsb[:], start=False, stop=True)

            out_sb = pp.tile([128, 2 * K], F32, tag="out")
            nc.scalar.copy(out_sb[:, 0:K], cntA[:])
            nc.scalar.copy(out_sb[:, K:2 * K], cntB[:])
            nc.sync.dma_start(counts[:], out_sb[:])

    nc.compile()
    return nc


def _host_inputs(fruits_prop, tools_prop, M_fruit, M_tool, M, domain_f, domain_t):
    G = np.ascontiguousarray(np.asarray(fruits_prop, np.float32)[np.asarray(domain_f, np.int64)])
    toolsD = np.asarray(tools_prop, np.float32)[np.asarray(domain_t, np.int64)]
    mf = np.ascontiguousarray(np.asarray(M_fruit, np.float32))
    mt = np.asarray(M_tool, np.float32)
    m = np.asarray(M, np.float32)
    WT = np.ascontiguousarray(((toolsD @ mt) @ m).T)   # [512, 256]
    in_maps = []
    for c in range(N_CORES):
        Gc = G[c * F_PER_CORE:(c + 1) * F_PER_CORE]
        in_maps.append({
            "g_t": np.ascontiguousarray(Gc.T),
            "mf": mf,
            "wt_in": WT,
        })
    return in_maps


def decode_counts(counts_sum):
    """counts_sum: [128, 512] summed over cores -> C[i, j] win counts (i < j)."""
    sign_rows = _sign_rows()
    C = np.zeros((K_DOMAIN, K_DOMAIN), np.float64)
    for j in range(1, K_DOMAIN):
        half = j // 128
        row = counts_sum[j % 128, half * K_DOMAIN: half * K_DOMAIN + j]
        if j in sign_rows:
            C[:j, j] = np.rint((row + BATCH) / 2.0)
        else:
            C[:j, j] = np.rint(row)
    return C


def _predict(C, domain_t, tools_labels):
    pos = np.full(N_TOOLS, -1, np.int64)
    pos[np.asarray(domain_t, np.int64)] = np.arange(K_DOMAIN)
    l1 = pos[np.asarray(tools_labels[0], np.int64)]
    l2 = pos[np.asarray(tools_labels[1], np.int64)]
    B = l1.shape[0]
    half = BATCH // 2
    choice = np.ones(B, np.int64)
    lt = (l1 >= 0) & (l2 >= 0) & (l1 < l2)
    gt = (l1 >= 0) & (l2 >= 0) & (l1 > l2)
    choice[lt] = np.where(C[l1[lt], l2[lt]] >= half, 0, 1)
    choice[gt] = np.where(C[l2[gt], l1[gt]] <= half, 0, 1)
    out = np.zeros((B, 4), np.float32)
    out[:, 0] = 1.0
    out[np.arange(B), 1 + choice] = 1.0
    return out


def kernel(fruits_prop, tools_prop, M_fruit, M_tool, M, min_r, domain_f,
           domain_t, tools_labels):
    global LAST_RESULTS
    in_maps = _host_inputs(fruits_prop, tools_prop, M_fruit, M_tool, M,
                           domain_f, domain_t)
    nc = _build_nc()
    res = run_bass_kernel_spmd(nc, in_maps, list(range(N_CORES)), trace=TRACE)
    LAST_RESULTS = res
    counts_sum = np.zeros((128, 2 * K_DOMAIN), np.float64)
    for c in range(N_CORES):
        counts_sum += res.results[c]["counts"].astype(np.float64)
    C = decode_counts(counts_sum)
    return _predict(C, domain_t, tools_labels)
